# revision 2
# baseline (speedup 1.0000x reference)
"""CaptionEmbedder kernel for Trainium2 (Bass/Tile), 8-core data-parallel.

Semantics (matching the reference):
    ent_idx  = clamp-to-49 of (caption_indices - 32000)   (oob -> 49)
    word_idx = caption_indices if < 32000 else pad_token
    out[b,l] = entities_encoded[b, ent_idx]  if caption_masks[b,l,0] == 1
               else word_embedding[word_idx]

Strategy: shard the batch dim (8 batches/core). Host concatenates the
core's entity shard [400, 512] onto the word table -> one combined table
[32400, 512] per core, so the device does a single fused gather:
  combined_row = mask ? (32000 + 50*local_b + ent_idx) : word_idx
The device computes combined_row with a handful of int32 vector ops and
then streams 2KB rows out of HBM with chunked indirect DMAs, overlapped
with contiguous stores of the output.
"""

import os
import sys
from functools import lru_cache

import numpy as np

for _p in ("/opt/trn_rl_repo",):
    if _p not in sys.path:
        sys.path.insert(0, _p)

# Problem shapes (hardcoded per contest contract).
V = 32000          # vocab size
B = 64             # batch
L = 200            # caption length
N_ENT = 50         # entities per batch
D = 512            # embedding dim
N_CORES = 8
B_LOC = B // N_CORES            # 8 batches per core
TOK = B_LOC * L                 # 1600 tokens per core
P = 128                         # SBUF partitions
COLS = -(-TOK // P)             # 13 columns of 128 tokens
TOK_PAD = P * COLS              # 1664
TBL = V + B_LOC * N_ENT         # 32400 rows in combined table

CHUNK = 3                       # gather-chunk width in columns


@lru_cache(maxsize=2)
def _build(pad_val: int, chunk: int = CHUNK):
    import concourse.bacc as bacc
    import concourse.bass as bass
    import concourse.tile as tile
    from concourse import mybir

    i32 = mybir.dt.int32
    f32 = mybir.dt.float32
    Op = mybir.AluOpType

    nc = bacc.Bacc("TRN2", target_bir_lowering=False, debug=False)

    tbl_h = nc.dram_tensor("table", [TBL, D], f32, kind="ExternalInput")
    idx_h = nc.dram_tensor("idx", [P, COLS], i32, kind="ExternalInput")
    msk_h = nc.dram_tensor("msk", [P, COLS], i32, kind="ExternalInput")
    ebs_h = nc.dram_tensor("ebase", [P, COLS], i32, kind="ExternalInput")
    out_h = nc.dram_tensor("out", [P, COLS, D], f32, kind="ExternalOutput")

    tbl_ap = tbl_h.ap()
    out_ap = out_h.ap()

    with tile.TileContext(nc) as tc:
        with (
            tc.tile_pool(name="small", bufs=1) as sp,
            tc.tile_pool(name="emb", bufs=4) as ep,
        ):
            idx = sp.tile([P, COLS], i32)
            msk = sp.tile([P, COLS], i32)
            ebs = sp.tile([P, COLS], i32)
            nc.sync.dma_start(out=idx[:], in_=idx_h.ap()[:, :])
            nc.sync.dma_start(out=msk[:], in_=msk_h.ap()[:, :])
            nc.sync.dma_start(out=ebs[:], in_=ebs_h.ap()[:, :])

            c49 = sp.tile([P, COLS], i32)
            cpad = sp.tile([P, COLS], i32)
            nc.vector.memset(c49[:], N_ENT - 1)
            nc.vector.memset(cpad[:], pad_val)

            # ent = min(idx - V, 49); where(ent < 0) -> 49
            ent = sp.tile([P, COLS], i32)
            nc.vector.tensor_scalar(ent[:], idx[:], V, N_ENT - 1,
                                    Op.subtract, Op.min)
            neg = sp.tile([P, COLS], i32)
            nc.vector.tensor_scalar(neg[:], ent[:], 0, None, Op.is_lt)
            nc.vector.copy_predicated(ent[:], neg[:], c49[:])
            # ent += 32000 + 50*local_b   (precomputed host-side in ebase)
            nc.vector.tensor_tensor(ent[:], ent[:], ebs[:], Op.add)

            # wsel = idx < V ? idx : pad
            isw = sp.tile([P, COLS], i32)
            nc.vector.tensor_scalar(isw[:], idx[:], V, None, Op.is_ge)
            wsel = sp.tile([P, COLS], i32)
            nc.vector.tensor_copy(wsel[:], idx[:])
            nc.vector.copy_predicated(wsel[:], isw[:], cpad[:])

            # comb = (msk == 1) ? ent : wsel
            eq1 = sp.tile([P, COLS], i32)
            nc.vector.tensor_scalar(eq1[:], msk[:], 1, None, Op.is_equal)
            comb = sp.tile([P, COLS], i32)
            nc.vector.tensor_copy(comb[:], wsel[:])
            nc.vector.copy_predicated(comb[:], eq1[:], ent[:])

            # Gather one 128-row column per indirect DMA (HW consumes one
            # offset per partition), store `chunk` columns per contiguous
            # DMA; the Tile scheduler pipelines gathers against stores.
            for c0 in range(0, COLS, chunk):
                cw = min(chunk, COLS - c0)
                emb = ep.tile([P, chunk * D], f32, tag="emb")
                for j in range(cw):
                    nc.gpsimd.indirect_dma_start(
                        out=emb[:, j * D : (j + 1) * D],
                        out_offset=None,
                        in_=tbl_ap[:, :],
                        in_offset=bass.IndirectOffsetOnAxis(
                            ap=comb[:, c0 + j : c0 + j + 1], axis=0
                        ),
                    )
                nc.sync.dma_start(
                    out=out_ap[:, c0 : c0 + cw, :], in_=emb[:, : cw * D]
                )

    nc.compile()
    return nc


def _pad_flat(a: np.ndarray) -> np.ndarray:
    out = np.zeros(TOK_PAD, dtype=a.dtype)
    out[:TOK] = a.reshape(-1)
    return out.reshape(P, COLS)


def _shard_inputs(caption_indices, entities_encoded, word_embedding,
                  caption_masks):
    caption_indices = np.asarray(caption_indices, dtype=np.int32)
    caption_masks = np.asarray(caption_masks, dtype=np.int32)
    entities_encoded = np.asarray(entities_encoded, dtype=np.float32)
    word_embedding = np.asarray(word_embedding, dtype=np.float32)

    ebase = (V + N_ENT * (np.arange(TOK) // L)).astype(np.int32)
    ebase = _pad_flat(ebase)

    in_maps = []
    for i in range(N_CORES):
        sl = slice(i * B_LOC, (i + 1) * B_LOC)
        tbl = np.concatenate(
            [word_embedding, entities_encoded[sl].reshape(B_LOC * N_ENT, D)],
            axis=0,
        )
        in_maps.append(
            {
                "table": np.ascontiguousarray(tbl),
                "idx": _pad_flat(caption_indices[sl]),
                "msk": _pad_flat(caption_masks[sl]),
                "ebase": ebase,
            }
        )
    return in_maps


LAST_RESULTS = None  # BassKernelResults of the most recent run (for test.py)


def kernel(caption_indices, entities_encoded, word_embedding, pad_token,
           caption_masks):
    global LAST_RESULTS
    from concourse.bass_utils import run_bass_kernel_spmd

    nc = _build(int(pad_token))
    in_maps = _shard_inputs(caption_indices, entities_encoded,
                            word_embedding, caption_masks)
    res = run_bass_kernel_spmd(
        nc,
        in_maps,
        list(range(N_CORES)),
        trace=bool(os.environ.get("CAPEMB_TRACE")),
    )
    LAST_RESULTS = res
    out = np.empty((B, L, D), dtype=np.float32)
    for i in range(N_CORES):
        out[i * B_LOC : (i + 1) * B_LOC] = (
            res.results[i]["out"].reshape(TOK_PAD, D)[:TOK].reshape(B_LOC, L, D)
        )
    return out
